# revision 1
# baseline (speedup 1.0000x reference)
"""3-layer GAT on 8 Trainium2 NeuronCores.

Strategy (per sharding hint): destination nodes + incident edges sharded
across 8 cores; weights replicated; per-layer AllGather of node features.

Per layer, per core:
  Phase A (dst side): for each 128-dst chunk, gather the chunk's feature
    rows, transpose on PE, matmul against [W_ad | W_skip] (+bias via ones
    row) -> a_d table (DRAM) + skip rows (SBUF).
  Phase B (edge side): edges sorted by dst, padded per chunk to x128.
    One indirect DMA gathers all source rows of a chunk ([128, T*F]).
    Per 128-edge tile: PE transpose -> xl matmul against
    [W | W@att_src] -> per-edge logits; leaky+exp; one-hot (is_equal of
    iota vs dst_local) aggregation matmul accumulates ex-weighted
    messages + denominators into PSUM; per chunk: normalize, add skip,
    ELU (layer 3: head-mean + log_softmax) -> output rows.

Softmax max-subtraction is dropped: logits here are O(10) so exp() is
exact-safe in fp32, and softmax is shift-invariant so results match the
reference to rounding.
"""
import os
import sys

for _p in ("/opt/trn_rl_repo", "/root/.axon_site/_ro/trn_rl_repo"):
    if os.path.isdir(_p) and _p not in sys.path:
        sys.path.insert(0, _p)

import numpy as np

N0, N1, N2, N3 = 131072, 32768, 8192, 2048
H, C_HID, C_OUT, F_IN = 4, 64, 47, 100
NC_ = 8
P = 128
F32 = None  # set after mybir import


# ---------------------------------------------------------------- host prep

def _build_schedule(src, dst, n_dst, n_src):
    """Per-core edge schedule: edges grouped by 128-dst chunk, then by
    32768-row source bucket (int16 dma_gather range), each bucket run
    padded to x128 slots. Slot k of a chunk = (tile k//128, partition
    k%128). Returns wrapped int16 gather indices + per-chunk-per-bucket
    tile counts (uniform across cores)."""
    nbk = -(-n_src // 32768)
    nd_core = n_dst // NC_
    n_chunks = nd_core // P
    per_core = []
    for c in range(NC_):
        base = c * nd_core
        m = (dst >= base) & (dst < base + nd_core)
        e_src = src[m].astype(np.int64)
        e_dst = (dst[m] - base).astype(np.int64)
        bk = e_src >> 15
        order = np.lexsort((bk, e_dst // P))
        e_src, e_dst, bk = e_src[order], e_dst[order], bk[order]
        counts = np.zeros((n_chunks, nbk), np.int64)
        np.add.at(counts, ((e_dst // P), bk), 1)
        per_core.append((e_src, e_dst, bk, counts))
    counts_all = np.stack([pc[3] for pc in per_core])            # [cores,ch,bk]
    tiles_pcb = -(-counts_all.max(axis=0) // P)                  # [ch, bk]
    # ensure at least one tile per chunk (empty chunks get one bucket-0 tile)
    for i in range(n_chunks):
        if tiles_pcb[i].sum() == 0:
            tiles_pcb[i, 0] = 1
    T_tot = int(tiles_pcb.sum())
    scheds = []
    for c in range(NC_):
        e_src, e_dst, bk, counts = per_core[c]
        idx16 = np.zeros(T_tot * P, dtype=np.int16)
        dloc = np.full(T_tot * P, -1.0, dtype=np.float32)
        adix = np.zeros(T_tot * P, dtype=np.int16)
        s = 0
        epos = 0
        for i in range(n_chunks):
            for b in range(nbk):
                n_e = int(counts[i, b])
                sl = slice(s * P, s * P + n_e)
                idx16[sl] = (e_src[epos:epos + n_e] - (b << 15)).astype(np.int16)
                dloc[sl] = (e_dst[epos:epos + n_e] - i * P).astype(np.float32)
                adix[sl] = e_dst[epos:epos + n_e].astype(np.int16)
                epos += n_e
                s += int(tiles_pcb[i, b])
        assert epos == len(e_src) and s == T_tot
        idxw = np.tile(idx16.reshape(T_tot * 8, 16).T, (8, 1))   # [128, T*8]
        adxw = np.tile(adix.reshape(T_tot * 8, 16).T, (8, 1))
        scheds.append(dict(
            idxw=np.ascontiguousarray(idxw),
            adxw=np.ascontiguousarray(adxw),
            dloc=np.ascontiguousarray(dloc.reshape(T_tot, P).T),
        ))
    return scheds, tiles_pcb.tolist(), n_chunks


def _fold(W, a_s, a_d, b, Ws, bs, Fpad):
    """W_aug [Fpad, cg+4]; W_bigA [Fpad, 4+Cs]; bias row [1, 4+Cs]."""
    h, c = a_s.shape
    F, cg = W.shape
    W_as = np.zeros((F, h), np.float32)
    W_ad = np.zeros((F, h), np.float32)
    for hh in range(h):
        W_as[:, hh] = W[:, hh * c:(hh + 1) * c] @ a_s[hh]
        W_ad[:, hh] = W[:, hh * c:(hh + 1) * c] @ a_d[hh]
    W_aug = np.zeros((Fpad, cg + h), np.float32)
    W_aug[:F, :cg] = W
    W_aug[:F, cg:] = W_as
    Cs = Ws.shape[1]
    W_big = np.zeros((Fpad, h + Cs), np.float32)
    W_big[:F, :h] = W_ad
    W_big[:F, h:] = Ws
    bias = np.zeros((1, h + Cs), np.float32)
    bias[0, h:] = b + bs
    return W_aug, W_big, bias


# ---------------------------------------------------------------- bass build

def _build_nc(cfg):
    from concourse import bass, bacc, mybir, tile
    from concourse.masks import make_identity
    f32 = mybir.dt.float32
    i32 = mybir.dt.int32
    i16 = mybir.dt.int16
    AF = mybir.ActivationFunctionType
    OP = mybir.AluOpType

    nc = bacc.Bacc("TRN2", target_bir_lowering=False, debug=False,
                   num_devices=NC_)

    # ---- I/O declarations
    x_pad = nc.declare_dram_parameter("x_pad", [N0, P], f32, isOutput=False)
    iota_in = nc.declare_dram_parameter("iota_f", [P, P], f32, isOutput=False)
    L = []
    for li, lc in enumerate(cfg["layers"]):
        d = {}
        T_tot, nch = lc["T_tot"], lc["nch"]
        d["idx"] = nc.declare_dram_parameter(f"idx{li}", [P, T_tot * 8], i16, isOutput=False)
        d["dloc"] = nc.declare_dram_parameter(f"dloc{li}", [P, T_tot], f32, isOutput=False)
        d["adix"] = nc.declare_dram_parameter(f"adix{li}", [P, T_tot * 8], i16, isOutput=False)
        d["dstrow"] = nc.declare_dram_parameter(f"dstrow{li}", [P, nch], i32, isOutput=False)
        Fp, Ca, Cpa = lc["Fpad"], lc["Ca"], lc["Cpa"]
        d["waug"] = nc.declare_dram_parameter(f"waug{li}", [Fp, Ca], f32, isOutput=False)
        d["wbig"] = nc.declare_dram_parameter(f"wbig{li}", [Fp, Cpa], f32, isOutput=False)
        d["brow"] = nc.declare_dram_parameter(f"brow{li}", [1, Cpa], f32, isOutput=False)
        L.append(d)
    out_d = nc.declare_dram_parameter("out", [N3 // NC_, C_OUT], f32, isOutput=True)

    with tile.TileContext(nc) as tc:
        with (
            tc.tile_pool(name="const", bufs=1) as constp,
            tc.tile_pool(name="persist", bufs=1) as perp,
            tc.tile_pool(name="g", bufs=2) as gp,
            tc.tile_pool(name="st", bufs=2) as stp,
            tc.tile_pool(name="m", bufs=2) as mp,
            tc.tile_pool(name="gt", bufs=6) as gtp,
            tc.tile_pool(name="small", bufs=2) as smp,
            tc.tile_pool(name="ppa", bufs=2, space="PSUM") as ppa,
            tc.tile_pool(name="ptp", bufs=2, space="PSUM") as ptp,
            tc.tile_pool(name="pxl", bufs=2, space="PSUM") as pxl,
            tc.tile_pool(name="pagg", bufs=2, space="PSUM") as pagg,
            tc.tile_pool(name="dram", bufs=1, space="DRAM") as dramp,
        ):
            ident = constp.tile([P, P], f32, tag="ident")
            make_identity(nc, ident[:])
            iota0 = constp.tile([P, P], f32, tag="iota0")
            nc.sync.dma_start(out=iota0[:], in_=iota_in[:, :])
            iota = constp.tile([P, P], f32, tag="iota")
            nc.vector.tensor_copy(out=iota[:], in_=iota0[:])
            ones = constp.tile([1, P], f32, tag="ones")
            nc.vector.memset(ones[0:1, :], 1.0)

            # persistent per-layer consts
            lt = []
            for li, lc in enumerate(cfg["layers"]):
                Fp, Ca, Cpa, Fk = lc["Fpad"], lc["Ca"], lc["Cpa"], lc["Fk"]
                dd = {}
                dd["waug"] = [constp.tile([P, Ca], f32, tag=f"waug{li}_{k}", name=f"waug{li}_{k}") for k in range(Fk)]
                for k in range(Fk):
                    nc.sync.dma_start(out=dd["waug"][k][:], in_=L[li]["waug"][k * P:(k + 1) * P, :])
                dd["wbig"] = [constp.tile([P, Cpa], f32, tag=f"wbig{li}_{k}", name=f"wbig{li}_{k}") for k in range(Fk)]
                for k in range(Fk):
                    nc.sync.dma_start(out=dd["wbig"][k][:], in_=L[li]["wbig"][k * P:(k + 1) * P, :])
                dd["brow"] = constp.tile([1, Cpa], f32, tag=f"brow{li}", name=f"brow{li}")
                nc.sync.dma_start(out=dd["brow"][0:1, :], in_=L[li]["brow"][0:1, :])
                nch, Cs = lc["nch"], lc["Cs"]
                dd["skip"] = perp.tile([P, nch * Cs], f32, tag=f"skip{li}", name=f"skip{li}")
                dd["adsb"] = perp.tile([P, nch * H], f32, tag=f"adsb{li}", name=f"adsb{li}")
                dd["ad_dram"] = dramp.tile([nch * P, 64], f32, tag=f"ad{li}", name=f"ad{li}")
                lt.append(dd)

            h1s = dramp.tile([N1 // NC_, 256], f32, tag="h1s")
            h1a = dramp.tile([N1, 256], f32, tag="h1a", addr_space="Shared")
            h2s = dramp.tile([N2 // NC_, 256], f32, tag="h2s")
            h2a = dramp.tile([N2, 256], f32, tag="h2a", addr_space="Shared")

            hsrc = [x_pad, h1a, h2a]
            hout = [(h1s, h1a), (h2s, h2a), (None, None)]

            for li, lc in enumerate(cfg["layers"]):
                Fp, Fk, cg, Ca, Cpa = lc["Fpad"], lc["Fk"], lc["cg"], lc["Ca"], lc["Cpa"]
                nch, Cs, tpc, tpcb = lc["nch"], lc["Cs"], lc["tpc"], lc["tpcb"]
                cgh = cg // H
                dd = lt[li]
                src_t = hsrc[li]
                src_ap = src_t[:, :] if li == 0 else src_t[:]

                # ---------------- phase A: dst-side (a_d table + skip rows)
                for i in range(nch):
                    dri = smp.tile([P, 1], i32, tag="dri")
                    nc.sync.dma_start(out=dri[:, 0:1], in_=L[li]["dstrow"][:, i:i + 1])
                    hd0 = gtp.tile([P, Fp], f32, tag="hd0")
                    nc.gpsimd.indirect_dma_start(
                        out=hd0[:, :],
                        out_offset=None,
                        in_=src_ap,
                        in_offset=bass.IndirectOffsetOnAxis(
                            ap=dri[:, 0:1], axis=0),
                    )

                    pa = ppa.tile([P, Cpa], f32, tag="pa", space="PSUM")
                    for k in range(Fk):
                        tp = ptp.tile([P, P], f32, tag="tp", space="PSUM")
                        nc.tensor.transpose(out=tp[:], in_=hd0[:, k * P:(k + 1) * P],
                                            identity=ident[:])
                        ht = gtp.tile([P, P], f32, tag="ht")
                        nc.vector.tensor_copy(out=ht[:], in_=tp[:])
                        nc.tensor.matmul(out=pa[:, :Cpa], lhsT=ht[:],
                                         rhs=dd["wbig"][k][:],
                                         start=(k == 0), stop=False)
                    nc.tensor.matmul(out=pa[:, :Cpa], lhsT=ones[0:1, :],
                                     rhs=dd["brow"][0:1, :], start=False, stop=True)
                    nc.vector.tensor_copy(out=dd["adsb"][:, i * H:(i + 1) * H],
                                          in_=pa[:, 0:H])
                    nc.vector.tensor_copy(out=dd["skip"][:, i * Cs:(i + 1) * Cs],
                                          in_=pa[:, H:H + Cs])
                # a_d table to DRAM: [nd, 4] <- [P, nch, 4]
                ad_view = dd["ad_dram"][:].rearrange("(c p) f -> p c f", p=P)[:, :, 0:H]
                nc.sync.dma_start(out=ad_view,
                                  in_=dd["adsb"][:].rearrange("p (c f) -> p c f", f=H))

                # ---------------- phase B: edges
                T_MAX = max(tpc)
                t0 = 0
                for i in range(nch):
                    T = tpc[i]
                    ixw = smp.tile([P, T_MAX * 8], i16, tag="ixw")
                    nc.sync.dma_start(out=ixw[:, :T * 8],
                                      in_=L[li]["idx"][:, 8 * t0:8 * (t0 + T)])
                    axw = smp.tile([P, T_MAX * 8], i16, tag="axw")
                    nc.sync.dma_start(out=axw[:, :T * 8],
                                      in_=L[li]["adix"][:, 8 * t0:8 * (t0 + T)])
                    g0 = gp.tile([P, T_MAX * Fp], f32, tag="g0")
                    off = 0
                    n_rows = [N0, N1, N2][li]
                    for b in range(len(tpcb[i])):
                        Tb = tpcb[i][b]
                        if Tb == 0:
                            continue
                        lo = b * 32768
                        hi = min(lo + 32768, n_rows)
                        src_sl = (src_t[lo:hi, :] if li == 0
                                  else src_t[:][lo:hi, :])
                        for s0 in range(0, Tb, 8):
                            sn = min(8, Tb - s0)
                            o2 = off + s0
                            nc.gpsimd.dma_gather(
                                out_ap=g0[:, o2 * Fp:(o2 + sn) * Fp].rearrange(
                                    "p (j r) -> p j r", r=Fp),
                                in_ap=src_sl,
                                idxs_ap=ixw[:, 8 * o2:8 * (o2 + sn)],
                                num_idxs=sn * P, num_idxs_reg=sn * P,
                                elem_size=Fp, single_packet=False)
                        off += Tb
                    adg0 = smp.tile([P, T_MAX * 64], f32, tag="adg0")
                    for s0 in range(0, T, 16):
                        sn = min(16, T - s0)
                        nc.gpsimd.dma_gather(
                            out_ap=adg0[:, s0 * 64:(s0 + sn) * 64].rearrange(
                                "p (j r) -> p j r", r=64),
                            in_ap=dd["ad_dram"][:],
                            idxs_ap=axw[:, 8 * s0:8 * (s0 + sn)],
                            num_idxs=sn * P, num_idxs_reg=sn * P,
                            elem_size=64, single_packet=False)
                    dl = smp.tile([P, T_MAX], f32, tag="dl")
                    nc.sync.dma_start(out=dl[:, :T], in_=L[li]["dloc"][:, t0:t0 + T])

                    # one-hot S_T [e, d] for all tiles of the chunk
                    st = stp.tile([P, T_MAX * P], f32, tag="st")
                    nc.vector.tensor_tensor(
                        out=st[:, :T * P].rearrange("p (t d) -> p t d", d=P),
                        in0=iota[:].rearrange("p (o d) -> p o d", o=1).to_broadcast([P, T, P]),
                        in1=dl[:, :T].rearrange("p (t o) -> p t o", o=1).to_broadcast([P, T, P]),
                        op=OP.is_equal,
                    )
                    m_all = mp.tile([P, T_MAX * Ca], f32, tag="m")
                    tb = smp.tile([P, T_MAX * H], f32, tag="tb")
                    lk = smp.tile([P, T_MAX * H], f32, tag="lk")
                    agg = pagg.tile([P, Ca], f32, tag="agg", space="PSUM")
                    for t in range(T):
                        xl = pxl.tile([P, Ca], f32, tag="xl", space="PSUM")
                        for k in range(Fk):
                            tp = ptp.tile([P, P], f32, tag="tp", space="PSUM")
                            nc.tensor.transpose(
                                out=tp[:], in_=g0[:, t * Fp + k * P: t * Fp + (k + 1) * P],
                                identity=ident[:])
                            gt = gtp.tile([P, P], f32, tag="gt")
                            nc.scalar.activation(out=gt[:], in_=tp[:], func=AF.Copy)
                            nc.tensor.matmul(out=xl[:, :Ca], lhsT=gt[:],
                                             rhs=dd["waug"][k][:],
                                             start=(k == 0), stop=(k == Fk - 1))
                        # logits = leaky(a_s(src) + a_d(dst)); ex into M cols cg:cg+4
                        ts_ = slice(t * H, (t + 1) * H)
                        nc.vector.tensor_tensor(
                            out=tb[:, ts_], in0=xl[:, cg:cg + H],
                            in1=adg0[:, t * 64:t * 64 + H], op=OP.add)
                        nc.vector.tensor_scalar(out=lk[:, ts_], in0=tb[:, ts_],
                                                scalar1=0.2, scalar2=None, op0=OP.mult)
                        nc.vector.tensor_tensor(out=lk[:, ts_], in0=lk[:, ts_],
                                                in1=tb[:, ts_], op=OP.max)
                        nc.scalar.activation(
                            out=m_all[:, t * Ca + cg:t * Ca + cg + H],
                            in_=lk[:, ts_], func=AF.Exp)
                        # M[:, :cg] = xl * ex (per-head broadcast)
                        nc.vector.tensor_tensor(
                            out=m_all[:, t * Ca:t * Ca + cg].rearrange(
                                "p (h c) -> p h c", c=cgh),
                            in0=xl[:, 0:cg].rearrange("p (h c) -> p h c", c=cgh),
                            in1=m_all[:, t * Ca + cg:t * Ca + cg + H].rearrange(
                                "p (h o) -> p h o", o=1).to_broadcast([P, H, cgh]),
                            op=OP.mult)
                        nc.tensor.matmul(out=agg[:, :Ca],
                                         lhsT=st[:, t * P:(t + 1) * P],
                                         rhs=m_all[:, t * Ca:(t + 1) * Ca],
                                         start=(t == 0), stop=(t == T - 1))
                    # ---- finalize chunk
                    rc = smp.tile([P, H], f32, tag="rc")
                    nc.vector.reciprocal(out=rc[:, :], in_=agg[:, cg:cg + H])
                    if li < 2:
                        ho = gp.tile([P, cg], f32, tag="ho")
                        for hh in range(H):
                            nc.vector.tensor_scalar(
                                out=ho[:, hh * cgh:(hh + 1) * cgh],
                                in0=agg[:, hh * cgh:(hh + 1) * cgh],
                                scalar1=rc[:, hh:hh + 1], scalar2=None, op0=OP.mult)
                        nc.vector.tensor_tensor(out=ho[:], in0=ho[:],
                                                in1=dd["skip"][:, i * Cs:(i + 1) * Cs],
                                                op=OP.add)
                        # ELU = relu(x) + exp(min(x,0)) - 1
                        mn = gp.tile([P, cg], f32, tag="mn")
                        nc.vector.tensor_scalar(out=mn[:], in0=ho[:], scalar1=0.0,
                                                scalar2=None, op0=OP.min)
                        nc.scalar.activation(out=mn[:], in_=mn[:], func=AF.Exp)
                        nc.scalar.activation(out=ho[:], in_=ho[:], func=AF.Relu)
                        nc.vector.tensor_tensor(out=ho[:], in0=ho[:], in1=mn[:], op=OP.add)
                        nc.vector.tensor_scalar(out=ho[:], in0=ho[:], scalar1=-1.0,
                                                scalar2=None, op0=OP.add)
                        hs = hout[li][0]
                        nc.sync.dma_start(out=hs[i * P:(i + 1) * P, :], in_=ho[:])
                    else:
                        # head mean + skip + log_softmax
                        hm = smp.tile([P, 4 * C_OUT], f32, tag="hm")
                        for hh in range(H):
                            nc.vector.tensor_scalar(
                                out=hm[:, hh * C_OUT:(hh + 1) * C_OUT],
                                in0=agg[:, hh * cgh:(hh + 1) * cgh],
                                scalar1=rc[:, hh:hh + 1], scalar2=0.25,
                                op0=OP.mult, op1=OP.mult)
                        ho = smp.tile([P, C_OUT], f32, tag="ho3")
                        nc.vector.tensor_tensor(out=ho[:], in0=hm[:, 0:C_OUT],
                                                in1=hm[:, C_OUT:2 * C_OUT], op=OP.add)
                        nc.vector.tensor_tensor(out=ho[:], in0=ho[:],
                                                in1=hm[:, 2 * C_OUT:3 * C_OUT], op=OP.add)
                        nc.vector.tensor_tensor(out=ho[:], in0=ho[:],
                                                in1=hm[:, 3 * C_OUT:4 * C_OUT], op=OP.add)
                        nc.vector.tensor_tensor(out=ho[:], in0=ho[:],
                                                in1=dd["skip"][:, i * Cs:(i + 1) * Cs],
                                                op=OP.add)
                        mx = smp.tile([P, 1], f32, tag="mx")
                        nc.vector.tensor_reduce(out=mx[:, 0:1], in_=ho[:],
                                                axis=mybir.AxisListType.X, op=OP.max)
                        z = smp.tile([P, C_OUT], f32, tag="z")
                        nc.vector.tensor_scalar(out=z[:], in0=ho[:],
                                                scalar1=mx[:, 0:1], scalar2=None,
                                                op0=OP.subtract)
                        ez = smp.tile([P, C_OUT], f32, tag="ez")
                        nc.scalar.activation(out=ez[:], in_=z[:], func=AF.Exp)
                        sm = smp.tile([P, 1], f32, tag="sm")
                        nc.vector.tensor_reduce(out=sm[:, 0:1], in_=ez[:],
                                                axis=mybir.AxisListType.X, op=OP.add)
                        ln = smp.tile([P, 1], f32, tag="ln")
                        nc.scalar.activation(out=ln[:, 0:1], in_=sm[:, 0:1], func=AF.Ln)
                        zo = smp.tile([P, C_OUT], f32, tag="zo")
                        nc.vector.tensor_scalar(out=zo[:], in0=z[:],
                                                scalar1=ln[:, 0:1], scalar2=None,
                                                op0=OP.subtract)
                        nc.sync.dma_start(out=out_d[i * P:(i + 1) * P, :], in_=zo[:])
                    t0 += T

                # ---------------- all-gather H for next layer
                if li < 2:
                    hs, ha = hout[li]
                    nc.gpsimd.collective_compute(
                        "AllGather",
                        mybir.AluOpType.bypass,
                        replica_groups=[list(range(NC_))],
                        ins=[hs[:].opt()],
                        outs=[ha[:].opt()],
                    )
    nc.compile()
    return nc


# ---------------------------------------------------------------- entry

def kernel(**inputs):
    out, _ = run(inputs, trace=False)
    return out


def run(inputs, trace=False):
    from concourse import bass_utils

    x = inputs["x"].astype(np.float32)
    x_pad = np.zeros((N0, P), np.float32)
    x_pad[:, :F_IN] = x

    sch1, tpcb1, nch1 = _build_schedule(inputs["src1"], inputs["dst1"], N1, N0)
    sch2, tpcb2, nch2 = _build_schedule(inputs["src2"], inputs["dst2"], N2, N1)
    sch3, tpcb3, nch3 = _build_schedule(inputs["src3"], inputs["dst3"], N3, N2)
    tpc1 = [sum(r) for r in tpcb1]
    tpc2 = [sum(r) for r in tpcb2]
    tpc3 = [sum(r) for r in tpcb3]

    Waug1, Wbig1, brow1 = _fold(inputs["W1"], inputs["as1"], inputs["ad1"],
                                inputs["b1"], inputs["Ws1"], inputs["bs1"], 128)
    Waug2, Wbig2, brow2 = _fold(inputs["W2"], inputs["as2"], inputs["ad2"],
                                inputs["b2"], inputs["Ws2"], inputs["bs2"], 256)
    Waug3, Wbig3, brow3 = _fold(inputs["W3"], inputs["as3"], inputs["ad3"],
                                inputs["b3"], inputs["Ws3"], inputs["bs3"], 256)

    cfg = {"layers": [
        dict(T_tot=sum(tpc1), nch=nch1, tpc=tpc1, tpcb=tpcb1, Fpad=128, Fk=1,
             cg=256, Ca=260, Cpa=260, Cs=256),
        dict(T_tot=sum(tpc2), nch=nch2, tpc=tpc2, tpcb=tpcb2, Fpad=256, Fk=2,
             cg=256, Ca=260, Cpa=260, Cs=256),
        dict(T_tot=sum(tpc3), nch=nch3, tpc=tpc3, tpcb=tpcb3, Fpad=256, Fk=2,
             cg=188, Ca=192, Cpa=51, Cs=47),
    ]}

    nc = _build_nc(cfg)

    iota_f = np.tile(np.arange(P, dtype=np.float32)[None, :], (P, 1))
    in_maps = []
    for c in range(NC_):
        m = {
            "x_pad": x_pad, "iota_f": iota_f,
            "waug0": Waug1, "wbig0": Wbig1, "brow0": brow1,
            "waug1": Waug2, "wbig1": Wbig2, "brow1": brow2,
            "waug2": Waug3, "wbig2": Wbig3, "brow2": brow3,
        }
        for li, (sch, nch, ndst) in enumerate(
                [(sch1, nch1, N1), (sch2, nch2, N2), (sch3, nch3, N3)]):
            s = sch[c]
            m[f"idx{li}"] = s["idxw"]
            m[f"dloc{li}"] = s["dloc"]
            m[f"adix{li}"] = s["adxw"]
            base = c * (ndst // NC_)
            m[f"dstrow{li}"] = np.ascontiguousarray(
                (base + np.arange(nch)[None, :] * P
                 + np.arange(P)[:, None]).astype(np.int32))
        in_maps.append(m)

    if trace:
        out, times = _bench_pjrt(nc, in_maps, iters=4)
        return out, times
    res = bass_utils.run_bass_kernel_spmd(nc, in_maps, list(range(NC_)),
                                          trace=False)
    out = np.concatenate([res.results[c]["out"] for c in range(NC_)], axis=0)
    return out.astype(np.float32), res


def _bench_pjrt(nc, in_maps, iters=4):
    """Mirror bass2jax.run_bass_via_pjrt multi-core path, but keep inputs
    device-resident and time repeated executions (min wall over iters)."""
    import time
    import jax
    from jax.sharding import Mesh, PartitionSpec, NamedSharding
    from jax.experimental.shard_map import shard_map
    from concourse import bass2jax, mybir

    bass2jax.install_neuronx_cc_hook()
    pid_name = nc.partition_id_tensor.name if nc.partition_id_tensor else None
    in_names, out_names, out_avals, zero_outs = [], [], [], []
    for alloc in nc.m.functions[0].allocations:
        if not isinstance(alloc, mybir.MemoryLocationSet):
            continue
        name = alloc.memorylocations[0].name
        if alloc.kind == "ExternalInput":
            if name != pid_name:
                in_names.append(name)
        elif alloc.kind == "ExternalOutput":
            out_names.append(name)
            shape = tuple(alloc.tensor_shape)
            dtype = mybir.dt.np(alloc.dtype)
            out_avals.append(jax.core.ShapedArray(shape, dtype))
            zero_outs.append(np.zeros(shape, dtype))
    n_params = len(in_names)
    all_names = in_names + out_names
    if pid_name is not None:
        all_names = all_names + [pid_name]

    def _body(*args):
        operands = list(args)
        if pid_name is not None:
            operands.append(bass2jax.partition_id_tensor())
        outs = bass2jax._bass_exec_p.bind(
            *operands, out_avals=tuple(out_avals), in_names=tuple(all_names),
            out_names=tuple(out_names), lowering_input_output_aliases=(),
            sim_require_finite=True, sim_require_nnan=True, nc=nc)
        return tuple(outs)

    devices = jax.devices()[:NC_]
    mesh = Mesh(np.asarray(devices), ("core",))
    in_specs = (PartitionSpec("core"),) * (n_params + len(out_names))
    out_specs = (PartitionSpec("core"),) * len(out_names)
    donate = tuple(range(n_params, n_params + len(out_names)))
    sharded = jax.jit(
        shard_map(_body, mesh=mesh, in_specs=in_specs, out_specs=out_specs,
                  check_rep=False),
        donate_argnums=donate, keep_unused=True)
    sh = NamedSharding(mesh, PartitionSpec("core"))
    concat_in = [
        jax.device_put(
            np.concatenate([np.asarray(in_maps[c][n]) for c in range(NC_)],
                           axis=0), sh)
        for n in in_names]
    times = []
    out_arrs = None
    for _ in range(iters):
        concat_zeros = [
            np.zeros((NC_ * z.shape[0], *z.shape[1:]), z.dtype)
            for z in zero_outs]
        t0 = time.time()
        out_arrs = sharded(*concat_in, *concat_zeros)
        jax.block_until_ready(out_arrs)
        times.append(time.time() - t0)
    i = out_names.index("out")
    full = np.asarray(out_arrs[i])
    out = full.reshape(NC_, -1, full.shape[-1]).reshape(-1, full.shape[-1])
    return out.astype(np.float32), times



# revision 3
# speedup vs baseline: 52.1144x; 52.1144x over previous
"""3-layer GAT on 8 Trainium2 NeuronCores — v2 (fp16 datapath).

Strategy: dst-node + edge sharding across 8 cores; fp16 tables/messages with
fp32 PSUM accumulation (max rel err ~4e-3 vs fp32 reference per host sim).

Layer 1 (linear trick): gather raw [x|1|a_s] rows per edge; weighted
aggregation of raw features per dst chunk (agg = onehot^T @ (rows * ex)),
then ONE projection through W per chunk. a_s is device-computed per node
(xt build + AllGather). a_d per edge via a transposed-one-hot matmul
against the chunk's a_d table (no per-edge gather).

Layers 2/3 (projected tables): each core projects its OWN finalized rows
into p = [(xl_h|1)x4 | a_s] fp16 tables; AllGather; edge phase gathers
p rows directly — no per-tile transpose/projection matmuls at all.

One-hot aggregation: st (edge->dst, DVE is_equal from dloc) for the
aggregation matmul; stT (host-precomputed) for the a_d lookup matmul.

Softmax max-subtraction dropped (logits O(10), exp exact-safe in fp32;
softmax shift-invariant). Denominators ride along as the '|1' columns.
"""
import os
import sys

for _p in ("/opt/trn_rl_repo", "/root/.axon_site/_ro/trn_rl_repo"):
    if os.path.isdir(_p) and _p not in sys.path:
        sys.path.insert(0, _p)

import numpy as np

N0, N1, N2, N3 = 131072, 32768, 8192, 2048
H, C_HID, C_OUT, F_IN = 4, 64, 47, 100
NC_ = 8
P = 128

# table layouts
W1T = 128          # xt cols: [x(100) | 1 | a_s(4) | 0*23]
W2T = 384          # p1 cols: [(xl_h(64)|1)*4 =260 | a_s(4) | pad]
W3T = 256          # p2 cols: [(xl_h(47)|1)*4 =192 | a_s(4) | pad]
GCAP = 2048        # max idxs per dma_gather call


# ---------------------------------------------------------------- host prep

def _build_schedule(src, dst, n_dst, n_src, pieces=False):
    """Per-core edge schedule grouped by 128-dst chunk then source bucket,
    each bucket run padded to x128 slots. num_idxs per call is the max real
    count over cores (uniform IR); pad idxs gather row 0.

    pieces=False: buckets are contiguous 32768-row ranges of one table.
    pieces=True: buckets are AllGather piece tables — piece k holds global
    rows {c*shard + k*qrt + r} at piece-row c*qrt + r, so gathers can start
    as soon as piece k's collective lands.
    """
    if pieces:
        nbk = 4
        shard = n_src // NC_
        qrt = shard // 4
        bko = (src % shard) // qrt
        idxo = (src // shard) * qrt + (src % qrt)
    else:
        nbk = -(-n_src // 32768)
        bko = src >> 15
        idxo = src - (bko << 15)
    nd_core = n_dst // NC_
    n_chunks = nd_core // P
    per_core = []
    for c in range(NC_):
        base = c * nd_core
        m = (dst >= base) & (dst < base + nd_core)
        e_idx = idxo[m].astype(np.int64)
        e_dst = (dst[m] - base).astype(np.int64)
        bk = bko[m].astype(np.int64)
        order = np.lexsort((bk, e_dst // P))
        e_idx, e_dst, bk = e_idx[order], e_dst[order], bk[order]
        counts = np.zeros((n_chunks, nbk), np.int64)
        np.add.at(counts, ((e_dst // P), bk), 1)
        per_core.append((e_idx, e_dst, counts))
    counts_all = np.stack([pc[2] for pc in per_core])            # [cores,ch,bk]
    ncall_pcb = counts_all.max(axis=0)                           # [ch, bk]
    tiles_pcb = -(-ncall_pcb // P)
    for i in range(n_chunks):
        if tiles_pcb[i].sum() == 0:
            tiles_pcb[i, 0] = 1
    T_tot = int(tiles_pcb.sum())
    scheds = []
    for c in range(NC_):
        e_idx, e_dst, counts = per_core[c]
        idx16 = np.zeros(T_tot * P, dtype=np.int16)
        dloc = np.full(T_tot * P, -1.0, dtype=np.float16)
        s = 0
        epos = 0
        for i in range(n_chunks):
            for b in range(nbk):
                n_e = int(counts[i, b])
                sl = slice(s * P, s * P + n_e)
                idx16[sl] = e_idx[epos:epos + n_e].astype(np.int16)
                dloc[sl] = (e_dst[epos:epos + n_e] - i * P).astype(np.float16)
                epos += n_e
                s += int(tiles_pcb[i, b])
        assert epos == len(e_idx) and s == T_tot
        idxw = np.tile(idx16.reshape(T_tot * 8, 16).T, (8, 1))   # [128, T*8]
        stT = np.zeros((P, T_tot * P), np.float16)
        real = np.nonzero(dloc >= 0)[0]
        dl = dloc[real].astype(np.int64)
        stT[dl, real] = 1.0
        scheds.append(dict(
            idxw=np.ascontiguousarray(idxw),
            dloc=np.ascontiguousarray(dloc.reshape(T_tot, P).T),
            stT=np.ascontiguousarray(stT),
        ))
    return scheds, tiles_pcb.tolist(), ncall_pcb.tolist(), n_chunks


def _fold_a(W, a):
    h, c = a.shape
    out = np.zeros((W.shape[0], h), np.float32)
    for hh in range(h):
        out[:, hh] = W[:, hh * c:(hh + 1) * c] @ a[hh]
    return out


def _prep_weights(inputs):
    f16 = np.float16
    w = {}
    # xt build: a_s1 = [x|1] @ was1   (rows 0:100 = W1.as-fold, row 100 = 0)
    was1 = np.zeros((P, H), np.float32)
    was1[:F_IN] = _fold_a(inputs["W1"], inputs["as1"])
    w["was1"] = was1.astype(f16)
    # L1 phase A: [x|1|..] @ wbig1 -> [a_d(4) | skip(256)], bias in row 100
    wbig1 = np.zeros((P, 4 + 256), np.float32)
    wbig1[:F_IN, :4] = _fold_a(inputs["W1"], inputs["ad1"])
    wbig1[:F_IN, 4:] = inputs["Ws1"]
    wbig1[F_IN, 4:] = inputs["b1"] + inputs["bs1"]
    w["wbig1"] = wbig1.astype(f16)
    # L1 chunk projection: W1 [100, 256] padded
    wp1 = np.zeros((P, 256), np.float32)
    wp1[:F_IN] = inputs["W1"]
    w["wp1"] = wp1.astype(f16)
    # L1->p1 projection [256, 264]: [(W2_h|0)x4 | W2.as2-fold]; ones row sep.
    wp2 = np.zeros((256, 264), np.float32)
    for hh in range(H):
        wp2[:, hh * 65:hh * 65 + 64] = inputs["W2"][:, hh * 64:(hh + 1) * 64]
    wp2[:, 260:264] = _fold_a(inputs["W2"], inputs["as2"])
    w["wp2"] = wp2.astype(f16)
    brow2 = np.zeros((1, 264), np.float32)
    brow2[0, [64, 129, 194, 259]] = 1.0
    w["brow2"] = brow2.astype(f16)
    # L2 phase A: h1 @ wbig2 -> [a_d2(4) | skip2(256)], bias via ones row
    wbig2 = np.zeros((256, 260), np.float32)
    wbig2[:, :4] = _fold_a(inputs["W2"], inputs["ad2"])
    wbig2[:, 4:] = inputs["Ws2"]
    w["wbig2"] = wbig2.astype(f16)
    brow2b = np.zeros((1, 260), np.float32)
    brow2b[0, 4:] = inputs["b2"] + inputs["bs2"]
    w["brow2b"] = brow2b.astype(f16)
    # L2->p2 projection [256, 196]: [(W3_h|0)x4 | W3.as3-fold]
    wp3 = np.zeros((256, 196), np.float32)
    for hh in range(H):
        wp3[:, hh * 48:hh * 48 + 47] = inputs["W3"][:, hh * 47:(hh + 1) * 47]
    wp3[:, 192:196] = _fold_a(inputs["W3"], inputs["as3"])
    w["wp3"] = wp3.astype(f16)
    brow3 = np.zeros((1, 196), np.float32)
    brow3[0, [47, 95, 143, 191]] = 1.0
    w["brow3"] = brow3.astype(f16)
    # L3 phase A: h2 @ wbig3 -> [a_d3(4) | skip3(47)]
    wbig3 = np.zeros((256, 51), np.float32)
    wbig3[:, :4] = _fold_a(inputs["W3"], inputs["ad3"])
    wbig3[:, 4:] = inputs["Ws3"]
    w["wbig3"] = wbig3.astype(f16)
    brow3b = np.zeros((1, 51), np.float32)
    brow3b[0, 4:] = inputs["b3"] + inputs["bs3"]
    w["brow3b"] = brow3b.astype(f16)
    return w


# ---------------------------------------------------------------- bass build

def _build_nc(cfg, debug=False):
    from concourse import bass, bacc, mybir, tile
    from concourse.masks import make_identity
    f32 = mybir.dt.float32
    f16 = mybir.dt.float16
    i16 = mybir.dt.int16
    i32 = mybir.dt.int32
    AF = mybir.ActivationFunctionType
    OP = mybir.AluOpType

    nc = bacc.Bacc("TRN2", target_bir_lowering=False, debug=False,
                   num_devices=NC_)

    L = cfg["layers"]
    # ---- I/O declarations
    xbs = nc.declare_dram_parameter("xbs", [N0 // NC_, P], f16, isOutput=False)
    xbd = nc.declare_dram_parameter("xbd", [N1 // NC_, P], f16, isOutput=False)
    wt = {}
    for name, shape in [("was1", [P, H]), ("wbig1", [P, 260]),
                        ("wp1", [P, 256]), ("wp2", [256, 264]),
                        ("brow2", [1, 264]), ("wbig2", [256, 260]),
                        ("brow2b", [1, 260]), ("wp3", [256, 196]),
                        ("brow3", [1, 196]), ("wbig3", [256, 51]),
                        ("brow3b", [1, 51])]:
        wt[name] = nc.declare_dram_parameter(name, shape, f16, isOutput=False)
    par = []
    iota_in = nc.declare_dram_parameter("iota_f", [P, P], f16, isOutput=False)
    for li, lc in enumerate(L):
        d = {}
        T_tot = lc["T_tot"]
        d["idx"] = nc.declare_dram_parameter(f"idx{li}", [P, T_tot * 8], i16, isOutput=False)
        d["dloc"] = nc.declare_dram_parameter(f"dloc{li}", [P, T_tot], f16, isOutput=False)
        d["stT"] = nc.declare_dram_parameter(f"stT{li}", [P, T_tot * P], f16, isOutput=False)
        par.append(d)
    dri2 = nc.declare_dram_parameter("dri2", [P, L[1]["nch"]], i32, isOutput=False)
    dri3 = nc.declare_dram_parameter("dri3", [P, L[2]["nch"]], i32, isOutput=False)
    out_d = nc.declare_dram_parameter("out", [N3 // NC_, C_OUT], f32, isOutput=True)
    dbg = {}
    if debug:
        for nm, shape in [("dbg_xt", [N0 // NC_, P]),
                          ("dbg_ad1", [P, L[0]["nch"] * H]),
                          ("dbg_skip1", [P, L[0]["nch"] * 256]),
                          ("dbg_h1", [N1 // NC_, 256]),
                          ("dbg_p1", [N1 // NC_, W2T]),
                          ("dbg_ad2", [P, L[1]["nch"] * H]),
                          ("dbg_h2", [N2 // NC_, 256]),
                          ("dbg_eb1", [P, L[0]["T_tot"] * H]),
                          ("dbg_g1", [P, L[0]["T_tot"] * W1T])]:
            dbg[nm] = nc.declare_dram_parameter(nm, shape, mybir.dt.float16
                                                if nm != "dbg_eb1" else f32,
                                                isOutput=True)

    with tile.TileContext(nc) as tc:
        with (
            tc.tile_pool(name="const", bufs=1) as constp,
            tc.tile_pool(name="persist", bufs=1) as perp,
            tc.tile_pool(name="g", bufs=4) as gp,
            tc.tile_pool(name="st", bufs=3) as stp,
            tc.tile_pool(name="sT", bufs=3) as sTp,
            tc.tile_pool(name="sm", bufs=3) as smp,
            tc.tile_pool(name="mxp", bufs=2) as mxp,
            tc.tile_pool(name="fin", bufs=2) as finp,
            tc.tile_pool(name="pad", bufs=2, space="PSUM") as ppad,
            tc.tile_pool(name="pagg", bufs=2, space="PSUM") as pagg,
            tc.tile_pool(name="ptr", bufs=2, space="PSUM") as ptr,
            tc.tile_pool(name="pout", bufs=2, space="PSUM") as pout,
            tc.tile_pool(name="dram", bufs=1, space="DRAM") as dramp,
        ):
            ident = constp.tile([P, P], f16, tag="ident")
            make_identity(nc, ident[:])
            iota = constp.tile([P, P], f16, tag="iota")
            nc.sync.dma_start(out=iota[:], in_=iota_in[:, :])
            ones = constp.tile([1, P], f16, tag="ones")
            nc.vector.memset(ones[0:1, :], 1.0)
            ocol = constp.tile([P, 1], f32, tag="ocol")
            nc.vector.memset(ocol[:, 0:1], 1.0)
            mcol = constp.tile([P, 1], f32, tag="mcol")
            nc.vector.memset(mcol[:, 0:1], -1.0)

            wsb = {}
            for name, kch in [("was1", 1), ("wbig1", 1), ("wp1", 1),
                              ("wp2", 2), ("wbig2", 2), ("wp3", 2),
                              ("wbig3", 2)]:
                cols = wt[name].shape[1]
                wsb[name] = [constp.tile([P, cols], f16, tag=f"{name}_{k}",
                                         name=f"{name}_{k}")
                             for k in range(kch)]
                for k in range(kch):
                    nc.sync.dma_start(out=wsb[name][k][:],
                                      in_=wt[name][k * P:(k + 1) * P, :])
            for name in ["brow2", "brow2b", "brow3", "brow3b"]:
                cols = wt[name].shape[1]
                wsb[name] = constp.tile([1, cols], f16, tag=name, name=name)
                nc.sync.dma_start(out=wsb[name][0:1, :], in_=wt[name][0:1, :])

            # persistent per-layer dst-side tables
            lt = []
            for li, lc in enumerate(L):
                nch, Cs = lc["nch"], lc["Cs"]
                dd = {}
                dd["skip"] = perp.tile([P, nch * Cs], f16, tag=f"skip{li}",
                                       name=f"skip{li}")
                dd["adsb"] = perp.tile([P, nch * H], f16, tag=f"adsb{li}",
                                       name=f"adsb{li}")
                lt.append(dd)

            QRT = (N0 // NC_) // 4
            xt_own = [dramp.tile([QRT, P], f16, tag=f"xt_own{k}",
                                 name=f"xt_own{k}") for k in range(4)]
            xt_piece = [dramp.tile([QRT * NC_, P], f16, tag=f"xt_p{k}",
                                   name=f"xt_p{k}", addr_space="Shared")
                        for k in range(4)]
            h1_own = dramp.tile([N1 // NC_, 256], f16, tag="h1_own")
            h1a = dramp.tile([N1, 256], f16, tag="h1a", addr_space="Shared")
            QP1 = (N1 // NC_) // 4
            p1_own = [dramp.tile([QP1, W2T], f16, tag=f"p1_own{k}",
                                 name=f"p1_own{k}") for k in range(4)]
            p1_piece = [dramp.tile([QP1 * NC_, W2T], f16, tag=f"p1_p{k}",
                                   name=f"p1_p{k}", addr_space="Shared")
                        for k in range(4)]
            h2_own = dramp.tile([N2 // NC_, 256], f16, tag="h2_own")
            h2a = dramp.tile([N2, 256], f16, tag="h2a", addr_space="Shared")
            p2_own = dramp.tile([N2 // NC_, W3T], f16, tag="p2_own")
            p2a = dramp.tile([N2, W3T], f16, tag="p2a", addr_space="Shared")

            def transpose_to_sbuf(src_ap, tagbase):
                """PE-transpose an SBUF f16 [128, <=128] ap; return SBUF tile."""
                tp = ptr.tile([P, P], f16, tag="tp", space="PSUM")
                ncols = src_ap.shape[-1]
                nc.tensor.transpose(out=tp[0:ncols, :], in_=src_ap,
                                    identity=ident[:])
                ts = smp.tile([P, P], f16, tag=tagbase)
                nc.vector.tensor_copy(out=ts[0:ncols, :], in_=tp[0:ncols, :])
                return ts

            # ---------------- xt build: own shard rows + a_s -> xt_own pieces
            # batched 8 tiles per DMA round-trip; each 4096-row quarter is
            # AllGathered as its own piece so L1 gathers start on piece 0
            XB = 8
            n_per_q = QRT // (P * XB)
            for k in range(4):
                for i in range(n_per_q):
                    r0 = k * QRT + i * XB * P
                    t0 = gp.tile([P, XB * P], f16, tag="xtb")
                    nc.sync.dma_start(
                        out=t0[:].rearrange("p (j f) -> p j f", f=P),
                        in_=xbs[r0:r0 + XB * P, :].rearrange(
                            "(j p) f -> p j f", p=P))
                    for j in range(XB):
                        ts = transpose_to_sbuf(t0[:, j * P:(j + 1) * P], "xtT")
                        pas = ppad.tile([P, H], f32, tag="pad", space="PSUM")
                        nc.tensor.matmul(out=pas[:, :], lhsT=ts[:],
                                         rhs=wsb["was1"][0][:], start=True, stop=True)
                        nc.vector.tensor_copy(out=t0[:, j * P + 101:j * P + 101 + H],
                                              in_=pas[:, :])
                    nc.sync.dma_start(
                        out=xt_own[k][i * XB * P:(i + 1) * XB * P, :].rearrange(
                            "(j p) f -> p j f", p=P),
                        in_=t0[:].rearrange("p (j f) -> p j f", f=P))
                nc.gpsimd.collective_compute(
                    "AllGather", mybir.AluOpType.bypass,
                    replica_groups=[list(range(NC_))],
                    ins=[xt_own[k][:].opt()], outs=[xt_piece[k][:].opt()])

            # ---------------- L1 phase A from xbd (own dst rows, host param)
            for i in range(L[0]["nch"]):
                t0 = gp.tile([P, P], f16, tag="pha1")
                nc.sync.dma_start(out=t0[:], in_=xbd[i * P:(i + 1) * P, :])
                ts = transpose_to_sbuf(t0[:], "pha1T")
                pa = pout.tile([P, 264], f32, tag="pa", space="PSUM")
                nc.tensor.matmul(out=pa[:, 0:260], lhsT=ts[:],
                                 rhs=wsb["wbig1"][0][:], start=True, stop=True)
                nc.vector.tensor_copy(out=lt[0]["adsb"][:, i * H:(i + 1) * H],
                                      in_=pa[:, 0:4])
                nc.vector.tensor_copy(out=lt[0]["skip"][:, i * 256:(i + 1) * 256],
                                      in_=pa[:, 4:260])

            # ---------------- L2/L3 phase A source rows via indirect gather
            def phase_a(li, src_tile, dri, wbig, brow, Cs):
                lc = L[li]
                nch = lc["nch"]
                hd = perp.tile([P, nch * 256], f16, tag=f"phsrc{li}")
                dri_s = smp.tile([P, max(nch, 1)], i32, tag="dri")
                nc.sync.dma_start(out=dri_s[:, 0:nch], in_=dri[:, :])
                for i in range(nch):
                    nc.gpsimd.indirect_dma_start(
                        out=hd[:, i * 256:(i + 1) * 256],
                        out_offset=None,
                        in_=src_tile[:],
                        in_offset=bass.IndirectOffsetOnAxis(ap=dri_s[:, i:i + 1], axis=0),
                    )
                for i in range(nch):
                    pa = pout.tile([P, 264], f32, tag="pa", space="PSUM")
                    for k in range(2):
                        ts = transpose_to_sbuf(
                            hd[:, i * 256 + k * P: i * 256 + (k + 1) * P], "phT")
                        nc.tensor.matmul(out=pa[:, 0:4 + Cs], lhsT=ts[:],
                                         rhs=wbig[k][:], start=(k == 0), stop=False)
                    nc.tensor.matmul(out=pa[:, 0:4 + Cs], lhsT=ones[0:1, :],
                                     rhs=brow[0:1, :], start=False, stop=True)
                    nc.vector.tensor_copy(out=lt[li]["adsb"][:, i * H:(i + 1) * H],
                                          in_=pa[:, 0:4])
                    nc.vector.tensor_copy(out=lt[li]["skip"][:, i * Cs:(i + 1) * Cs],
                                          in_=pa[:, 4:4 + Cs])

            # ---------------- edge phase (shared across layers)
            def edge_layer(li, buckets, finalize):
                """buckets: list of (table, lo, hi) gather sources per bucket."""
                lc = L[li]
                nch, tpcb, ncpb = lc["nch"], lc["tpcb"], lc["ncpb"]
                E = lc["elem"]           # gather row width (cols of src_tab)
                CM = lc["cm"]            # m width per head incl |1 (65/101/48)
                NA = CM * H              # agg width
                aoff = lc["aoff"]        # a_s col offset in gathered row
                T_MAX = max(sum(r) for r in tpcb)
                dd = lt[li]
                t0 = 0
                for i in range(nch):
                    T = sum(tpcb[i])
                    ixw = smp.tile([P, T_MAX * 8], i16, tag="ixw")
                    nc.sync.dma_start(out=ixw[:, :T * 8],
                                      in_=par[li]["idx"][:, 8 * t0:8 * (t0 + T)])
                    dl = smp.tile([P, T_MAX], f16, tag="dl")
                    nc.sync.dma_start(out=dl[:, :T],
                                      in_=par[li]["dloc"][:, t0:t0 + T])
                    sT = sTp.tile([P, T_MAX * P], f16, tag="sT")
                    nc.sync.dma_start(out=sT[:, :T * P],
                                      in_=par[li]["stT"][:, P * t0:P * (t0 + T)])
                    st = stp.tile([P, T_MAX * P], f16, tag="st")
                    nc.vector.tensor_tensor(
                        out=st[:, :T * P].rearrange("p (t d) -> p t d", d=P),
                        in0=dl[:, :T].rearrange("p (t o) -> p t o", o=1).to_broadcast([P, T, P]),
                        in1=iota[:, :].rearrange("p (o d) -> p o d", o=1).to_broadcast([P, T, P]),
                        op=OP.is_equal)
                    g0 = gp.tile([P, T_MAX * E], f16, tag="g0")
                    if i < 4:
                        # first-use SBUF garbage (possibly NaN/Inf f16) must
                        # not reach exp/matmul via unfilled pad slots
                        nc.vector.memset(g0[:], 0.0)
                    off = 0
                    for b in range(len(tpcb[i])):
                        Tb = tpcb[i][b]
                        if Tb == 0:
                            continue
                        ncall = ncpb[i][b]
                        tab, lo, hi = buckets[b]
                        s0 = 0
                        while ncall > 0:
                            n_this = min(ncall, GCAP)
                            tile_span = -(-n_this // P)
                            o2 = off + s0
                            nc.gpsimd.dma_gather(
                                out_ap=g0[:, o2 * E:(o2 + tile_span) * E].rearrange(
                                    "p (j r) -> p j r", r=E),
                                in_ap=tab[lo:hi, :],
                                idxs_ap=ixw[:, 8 * o2:8 * o2 + (-(-n_this // 16))],
                                num_idxs=n_this, num_idxs_reg=n_this,
                                elem_size=E, single_packet=False)
                            ncall -= n_this
                            s0 += tile_span
                        off += Tb
                    # logits + messages in groups of GSZ tiles: few ops (cheap
                    # queues) but fine enough grain to pipeline across engines
                    GSZ = 6
                    padc = ppad.tile([P, T_MAX * H], f32, tag="pad", space="PSUM")
                    for t in range(T):
                        nc.tensor.matmul(out=padc[:, t * H:(t + 1) * H],
                                         lhsT=sT[:, t * P:(t + 1) * P],
                                         rhs=dd["adsb"][:, i * H:(i + 1) * H],
                                         start=True, stop=True)
                    eb = smp.tile([P, T_MAX * H], f32, tag="eb")
                    lk = smp.tile([P, T_MAX * H], f32, tag="lk")
                    exb = smp.tile([P, T_MAX * H], f32, tag="exb")
                    mxc = mxp.tile([P, T_MAX * NA], f16, tag="mx")
                    agg = pagg.tile([P, NA], f32, tag="agg", space="PSUM")
                    for gti in range(0, T, GSZ):
                        gn = min(GSZ, T - gti)
                        sl4 = slice(gti * H, (gti + gn) * H)
                        nc.vector.tensor_tensor(
                            out=eb[:, sl4].rearrange("p (t h) -> p t h", h=H),
                            in0=g0[:, gti * E:(gti + gn) * E].rearrange(
                                "p (t r) -> p t r", r=E)[:, :, aoff:aoff + H],
                            in1=padc[:, sl4].rearrange("p (t h) -> p t h", h=H),
                            op=OP.add)
                        nc.scalar.activation(out=lk[:, sl4], in_=eb[:, sl4],
                                             func=AF.Copy, scale=0.2)
                        nc.vector.tensor_tensor(out=eb[:, sl4], in0=eb[:, sl4],
                                                in1=lk[:, sl4], op=OP.max)
                        nc.scalar.activation(out=exb[:, sl4], in_=eb[:, sl4],
                                             func=AF.Exp)
                        if li == 0:
                            in0 = g0[:, gti * E:(gti + gn) * E].rearrange(
                                "p (t o r) -> p t o r", o=1, r=E)[:, :, :, 0:CM].to_broadcast(
                                [P, gn, H, CM])
                        else:
                            in0 = g0[:, gti * E:(gti + gn) * E].rearrange(
                                "p (t r) -> p t r", r=E)[:, :, 0:H * CM].rearrange(
                                "p t (h c) -> p t h c", c=CM)
                        nc.vector.tensor_tensor(
                            out=mxc[:, gti * NA:(gti + gn) * NA].rearrange(
                                "p (t h c) -> p t h c", c=CM, h=H),
                            in0=in0,
                            in1=exb[:, sl4].rearrange(
                                "p (t h o) -> p t h o", o=1, h=H).to_broadcast(
                                [P, gn, H, CM]),
                            op=OP.mult)
                        for t in range(gti, gti + gn):
                            nc.tensor.matmul(out=agg[:, :],
                                             lhsT=st[:, t * P:(t + 1) * P],
                                             rhs=mxc[:, t * NA:(t + 1) * NA],
                                             start=(t == 0), stop=(t == T - 1))
                    if debug and li == 0:
                        nc.sync.dma_start(out=dbg["dbg_eb1"][:, H * t0:H * (t0 + T)],
                                          in_=eb[:, :T * H])
                        nc.sync.dma_start(out=dbg["dbg_g1"][:, E * t0:E * (t0 + T)],
                                          in_=g0[:, :T * E])
                    finalize(i, agg)
                    t0 += T

            # ---------------- finalizers
            def proj_store(i, hc, wp, brow, pcols, owsel):
                """project finalized chunk rows into p table + store."""
                pp = pout.tile([P, 264], f32, tag="pa", space="PSUM")
                for k in range(2):
                    ts = transpose_to_sbuf(hc[:, k * P:(k + 1) * P], "pjT")
                    nc.tensor.matmul(out=pp[:, 0:pcols], lhsT=ts[:],
                                     rhs=wp[k][:], start=(k == 0), stop=False)
                nc.tensor.matmul(out=pp[:, 0:pcols], lhsT=ones[0:1, :],
                                 rhs=brow[0:1, :], start=False, stop=True)
                pc = finp.tile([P, pcols], f16, tag="pc")
                nc.vector.tensor_copy(out=pc[:], in_=pp[:, 0:pcols])
                own, r0 = owsel(i)
                nc.sync.dma_start(out=own[r0:r0 + P, 0:pcols], in_=pc[:])

            def fin_concat(li, CM, hown, owsel, wp, brow, pcols, post=None):
                dd = lt[li]

                def fin(i, agg):
                    ags = finp.tile([P, H * CM], f16, tag="ags")
                    nc.vector.tensor_copy(out=ags[:], in_=agg[:, :])
                    rc = finp.tile([P, H], f32, tag="rc")
                    nc.vector.reciprocal(
                        out=rc[:].rearrange("p (h o) -> p h o", o=1),
                        in_=ags[:].rearrange("p (h c) -> p h c", c=CM)[:, :, CM - 1:CM])
                    if li == 0:
                        # project aggregated raw features through W1 per head
                        po = pout.tile([P, 264], f32, tag="pa", space="PSUM")
                        for hh in range(H):
                            ts = transpose_to_sbuf(
                                ags[:, hh * CM:hh * CM + 100], "agT")
                            nc.tensor.matmul(
                                out=po[:, hh * 64:(hh + 1) * 64],
                                lhsT=ts[0:100, :],
                                rhs=wsb["wp1"][0][0:100, hh * 64:(hh + 1) * 64],
                                start=True, stop=True)
                        ho = finp.tile([P, 256], f32, tag="ho")
                        nc.vector.tensor_tensor(
                            out=ho[:].rearrange("p (h c) -> p h c", c=64),
                            in0=po[:, 0:256].rearrange("p (h c) -> p h c", c=64),
                            in1=rc[:].rearrange("p (h o) -> p h o", o=1).to_broadcast([P, H, 64]),
                            op=OP.mult)
                    else:
                        ho = finp.tile([P, 256], f32, tag="ho")
                        nc.vector.tensor_tensor(
                            out=ho[:].rearrange("p (h c) -> p h c", c=64),
                            in0=ags[:].rearrange("p (h c) -> p h c", c=CM)[:, :, 0:64],
                            in1=rc[:].rearrange("p (h o) -> p h o", o=1).to_broadcast([P, H, 64]),
                            op=OP.mult)
                    Cs = 256
                    nc.vector.tensor_tensor(out=ho[:, 0:Cs], in0=ho[:, 0:Cs],
                                            in1=dd["skip"][:, i * Cs:(i + 1) * Cs],
                                            op=OP.add)
                    # ELU(x) = relu(x) + min(exp(x), 1) - 1  (exp monotonic)
                    mn = finp.tile([P, 256], f32, tag="mn")
                    nc.scalar.activation(out=mn[:, 0:Cs], in_=ho[:, 0:Cs], func=AF.Exp)
                    nc.vector.tensor_tensor(out=mn[:, 0:Cs], in0=mn[:, 0:Cs],
                                            in1=ocol[:, 0:1].to_broadcast([P, Cs]),
                                            op=OP.min)
                    nc.scalar.activation(out=ho[:, 0:Cs], in_=ho[:, 0:Cs], func=AF.Relu)
                    nc.vector.tensor_tensor(out=ho[:, 0:Cs], in0=ho[:, 0:Cs],
                                            in1=mn[:, 0:Cs], op=OP.add)
                    hc = finp.tile([P, 256], f16, tag="hc")
                    nc.vector.tensor_tensor(out=hc[:, 0:Cs], in0=ho[:, 0:Cs],
                                            in1=mcol[:, 0:1].to_broadcast([P, Cs]),
                                            op=OP.add)
                    nc.sync.dma_start(out=hown[i * P:(i + 1) * P, :], in_=hc[:, 0:Cs])
                    proj_store(i, hc[:, 0:256], wp, brow, pcols, owsel)
                    if post is not None:
                        post(i)
                return fin

            def fin_l3(i, agg):
                CM, CH = 48, 47
                ags = finp.tile([P, H * CM], f16, tag="ags")
                nc.vector.tensor_copy(out=ags[:], in_=agg[:, :])
                rc = finp.tile([P, H], f32, tag="rc")
                nc.vector.reciprocal(
                    out=rc[:].rearrange("p (h o) -> p h o", o=1),
                    in_=ags[:].rearrange("p (h c) -> p h c", c=CM)[:, :, CH:CH + 1])
                hm = finp.tile([P, H * CH], f32, tag="hm")
                nc.vector.tensor_tensor(
                    out=hm[:].rearrange("p (h c) -> p h c", c=CH),
                    in0=ags[:].rearrange("p (h c) -> p h c", c=CM)[:, :, 0:CH],
                    in1=rc[:].rearrange("p (h o) -> p h o", o=1).to_broadcast([P, H, CH]),
                    op=OP.mult)
                ho = finp.tile([P, C_OUT], f32, tag="ho3")
                nc.vector.tensor_tensor(out=ho[:], in0=hm[:, 0:C_OUT],
                                        in1=hm[:, C_OUT:2 * C_OUT], op=OP.add)
                nc.vector.tensor_tensor(out=ho[:], in0=ho[:],
                                        in1=hm[:, 2 * C_OUT:3 * C_OUT], op=OP.add)
                nc.vector.tensor_tensor(out=ho[:], in0=ho[:],
                                        in1=hm[:, 3 * C_OUT:4 * C_OUT], op=OP.add)
                nc.vector.tensor_scalar(out=ho[:], in0=ho[:], scalar1=0.25,
                                        scalar2=None, op0=OP.mult)
                nc.vector.tensor_tensor(out=ho[:], in0=ho[:],
                                        in1=lt[2]["skip"][:, i * C_OUT:(i + 1) * C_OUT],
                                        op=OP.add)
                mx = finp.tile([P, 1], f32, tag="mx3")
                nc.vector.tensor_reduce(out=mx[:, 0:1], in_=ho[:],
                                        axis=mybir.AxisListType.X, op=OP.max)
                z = finp.tile([P, C_OUT], f32, tag="z3")
                nc.vector.tensor_tensor(
                    out=z[:], in0=ho[:],
                    in1=mx[:, 0:1].to_broadcast([P, C_OUT]), op=OP.subtract)
                ez = finp.tile([P, C_OUT], f32, tag="ez3")
                nc.scalar.activation(out=ez[:], in_=z[:], func=AF.Exp)
                sm = finp.tile([P, 1], f32, tag="sm3")
                nc.vector.tensor_reduce(out=sm[:, 0:1], in_=ez[:],
                                        axis=mybir.AxisListType.X, op=OP.add)
                ln = finp.tile([P, 1], f32, tag="ln3")
                nc.scalar.activation(out=ln[:, 0:1], in_=sm[:, 0:1], func=AF.Ln)
                zo = finp.tile([P, C_OUT], f32, tag="zo3")
                nc.vector.tensor_tensor(
                    out=zo[:], in0=z[:],
                    in1=ln[:, 0:1].to_broadcast([P, C_OUT]), op=OP.subtract)
                nc.sync.dma_start(out=out_d[i * P:(i + 1) * P, :], in_=zo[:])

            # ---------------- run layers
            def post1(i):
                # progressive p1 piece AllGathers hidden under L1 compute;
                # h1 goes first at the end (phase A needs it before edges
                # need piece 3)
                if i == L[0]["nch"] - 1:
                    nc.gpsimd.collective_compute(
                        "AllGather", mybir.AluOpType.bypass,
                        replica_groups=[list(range(NC_))],
                        ins=[h1_own[:].opt()], outs=[h1a[:].opt()])
                if i % 8 == 7:
                    k = i // 8
                    nc.gpsimd.collective_compute(
                        "AllGather", mybir.AluOpType.bypass,
                        replica_groups=[list(range(NC_))],
                        ins=[p1_own[k][:].opt()], outs=[p1_piece[k][:].opt()])

            fin1 = fin_concat(0, 101, h1_own,
                              lambda i: (p1_own[i // 8], (i % 8) * P),
                              wsb["wp2"], wsb["brow2"], 264, post=post1)
            if debug:
                for k in range(4):
                    nc.sync.dma_start(
                        out=dbg["dbg_xt"][k * QRT:(k + 1) * QRT, :],
                        in_=xt_own[k][:])
                nc.sync.dma_start(out=dbg["dbg_ad1"][:, :], in_=lt[0]["adsb"][:])
                nc.sync.dma_start(out=dbg["dbg_skip1"][:, :], in_=lt[0]["skip"][:])
            edge_layer(0, [(xt_piece[k], 0, QRT * NC_) for k in range(4)], fin1)

            if debug:
                nc.sync.dma_start(out=dbg["dbg_h1"][:, :], in_=h1_own[:])
                for k in range(4):
                    nc.sync.dma_start(
                        out=dbg["dbg_p1"][k * QP1:(k + 1) * QP1, :],
                        in_=p1_own[k][:])
            phase_a(1, h1a, dri2, wsb["wbig2"], wsb["brow2b"], 256)
            fin2 = fin_concat(1, 65, h2_own, lambda i: (p2_own, i * P),
                              wsb["wp3"], wsb["brow3"], 196)
            if debug:
                nc.sync.dma_start(out=dbg["dbg_ad2"][:, :], in_=lt[1]["adsb"][:])
            edge_layer(1, [(p1_piece[k], 0, QP1 * NC_) for k in range(4)], fin2)
            nc.gpsimd.collective_compute(
                "AllGather", mybir.AluOpType.bypass,
                replica_groups=[list(range(NC_))],
                ins=[h2_own[:].opt()], outs=[h2a[:].opt()])
            nc.gpsimd.collective_compute(
                "AllGather", mybir.AluOpType.bypass,
                replica_groups=[list(range(NC_))],
                ins=[p2_own[:].opt()], outs=[p2a[:].opt()])

            if debug:
                nc.sync.dma_start(out=dbg["dbg_h2"][:, :], in_=h2_own[:])
            phase_a(2, h2a, dri3, wsb["wbig3"], wsb["brow3b"], 47)
            edge_layer(2, [(p2a, 0, N2)], fin_l3)
    nc.compile()
    return nc


# ---------------------------------------------------------------- entry

def prepare(inputs, debug=False):
    x = np.asarray(inputs["x"], np.float32)
    sch1, tpcb1, ncpb1, nch1 = _build_schedule(
        np.asarray(inputs["src1"]), np.asarray(inputs["dst1"]), N1, N0,
        pieces=True)
    sch2, tpcb2, ncpb2, nch2 = _build_schedule(
        np.asarray(inputs["src2"]), np.asarray(inputs["dst2"]), N2, N1,
        pieces=True)
    sch3, tpcb3, ncpb3, nch3 = _build_schedule(
        np.asarray(inputs["src3"]), np.asarray(inputs["dst3"]), N3, N2)
    w = _prep_weights({k: np.asarray(v, np.float32) if v.dtype != np.int32 else v
                       for k, v in inputs.items()})

    cfg = {"layers": [
        dict(T_tot=sum(sum(r) for r in tpcb1), nch=nch1, tpcb=tpcb1,
             ncpb=ncpb1, elem=W1T, cm=101, ch=100, m_in=101, aoff=101, Cs=256),
        dict(T_tot=sum(sum(r) for r in tpcb2), nch=nch2, tpcb=tpcb2,
             ncpb=ncpb2, elem=W2T, cm=65, ch=64, m_in=260, aoff=260, Cs=256),
        dict(T_tot=sum(sum(r) for r in tpcb3), nch=nch3, tpcb=tpcb3,
             ncpb=ncpb3, elem=W3T, cm=48, ch=47, m_in=192, aoff=192, Cs=47),
    ]}
    nc = _build_nc(cfg, debug=debug)

    # host-side fp16 x table rows [x | 1 | (a_s placeholder) | 0]
    xb = np.zeros((N0, P), np.float16)
    xb[:, :F_IN] = x.astype(np.float16)
    xb[:, F_IN] = 1.0
    iota_f = np.tile(np.arange(P, dtype=np.float16)[None, :], (P, 1))

    in_maps = []
    for c in range(NC_):
        m = dict(w)
        m["xbs"] = np.ascontiguousarray(xb[c * (N0 // NC_):(c + 1) * (N0 // NC_)])
        m["xbd"] = np.ascontiguousarray(xb[c * (N1 // NC_):(c + 1) * (N1 // NC_)])
        m["iota_f"] = iota_f
        for li, sch in enumerate([sch1, sch2, sch3]):
            s = sch[c]
            m[f"idx{li}"] = s["idxw"]
            m[f"dloc{li}"] = s["dloc"]
            m[f"stT{li}"] = s["stT"]
        for li, (name, ndst, nch) in enumerate(
                [(None, None, None), ("dri2", N2, nch2), ("dri3", N3, nch3)]):
            if name is None:
                continue
            base = c * (ndst // NC_)
            m[name] = np.ascontiguousarray(
                (base + np.arange(nch)[None, :] * P
                 + np.arange(P)[:, None]).astype(np.int32))
        in_maps.append(m)
    return nc, in_maps


def kernel(**inputs):
    from concourse import bass_utils
    nc, in_maps = prepare(inputs)
    res = bass_utils.run_bass_kernel_spmd(nc, in_maps, list(range(NC_)),
                                          trace=False)
    out = np.concatenate([res.results[c]["out"] for c in range(NC_)], axis=0)
    return out.astype(np.float32)
